# revision 17
# baseline (speedup 1.0000x reference)
"""Trainium2 Bass kernel for nn_Attention_30666066493686.

Region-attention over N=36 regions:
  hidden = tanh(region @ Wr + frame @ Wf + b_att)          [T,N,B,A]
  att    = hidden . W_full  (+ b_full, dropped: softmax-shift invariant)
  alpha  = softmax_n(where(mask, -1e9, att))
  out    = sum_n alpha * region                            [T,B,D]

Sharding: data-parallel over T across 8 NeuronCores (4 timesteps each);
params replicated; no collectives.

Per-core dataflow (rows = (n,b) flattened = 2304 = 18 chunks of 128):
  - region loaded natural [rows, D] (one DMA per t)
  - PE transposes 128x128 blocks -> region^T chunks for phase 1
  - phase 1: hidden^T[A, rows] = Wr^T @ region^T + rank-extended bias
    (frame projection + b_att folded in as extra contraction rows vs a
    tiled I64 so no DVE bias add is needed)
  - att column-ized on PE (lhsT = tanh chunk, rhs = W_full) -> [rows, 1]
    so softmax runs partition-parallel
  - softmax without max-subtraction (|att| <= ~12, exp is safe); mask
    applied as a 0/1 multiply after exp; normalization folded into the
    output scale (out = (sum_n e_n * region_n) / S)
  - phase 2: out[b, D] = diag-expanded(exp att)^T @ region_natural on PE
"""

import numpy as np

T, N, B, D, A = 32, 36, 64, 512, 128
N_CORES = 8
T_LOC = T // N_CORES           # 4
ROWS = N * B                   # 2304
NCH = ROWS // 128              # 18
GROUPS = [(0, 512), (512, 512), (1024, 512), (1536, 512), (2048, 256)]

_NC_CACHE = {}


def _build_nc(iters=1):
    import concourse.bacc as bacc
    import concourse.bass as bass
    from concourse import mybir
    from concourse.tile import TileContext

    f32 = mybir.dt.float32
    AF = mybir.ActivationFunctionType

    f32r = mybir.dt.float32r

    def r(ap):
        # reinterpret an fp32 AP as float32r (for PSUM transpose outputs)
        return ap.bitcast(mybir.dt.float32r)

    nc = bacc.Bacc(
        "TRN2", target_bir_lowering=False, debug=False, num_devices=N_CORES
    )
    region = nc.dram_tensor("region", [T_LOC, N, B, D], f32r, kind="ExternalInput")
    frame = nc.dram_tensor("frame", [T_LOC, B, D], f32r, kind="ExternalInput")
    mask = nc.dram_tensor(
        "mask", [T_LOC, N, B], mybir.dt.uint8, kind="ExternalInput"
    )
    watt = nc.dram_tensor("watt", [2 * D, A], f32r, kind="ExternalInput")
    wfull = nc.dram_tensor("wfull", [A, 1], f32, kind="ExternalInput")
    batt = nc.dram_tensor("batt", [1, A], f32, kind="ExternalInput")
    ident = nc.dram_tensor("ident", [128, 128], f32r, kind="ExternalInput")
    ident18 = nc.dram_tensor("ident18", [18, 18], f32, kind="ExternalInput")
    diag01 = nc.dram_tensor("diag01", [128, 64], f32, kind="ExternalInput")
    i64 = nc.dram_tensor("i64", [128, 64], f32r, kind="ExternalInput")
    ones_row = nc.dram_tensor("ones_row", [1, 128], f32, kind="ExternalInput")
    out = nc.dram_tensor("out", [T_LOC, B, D], f32, kind="ExternalOutput")

    with TileContext(nc) as tc:
        with (
            tc.tile_pool(name="consts", bufs=1) as consts,
            tc.tile_pool(name="rnatp", bufs=2) as rnatp,
            tc.tile_pool(name="rtp", bufs=3) as rtp,
            tc.tile_pool(name="tanhp", bufs=2) as tanhp,
            tc.tile_pool(name="smallp", bufs=2) as smallp,
            tc.tile_pool(name="diagp", bufs=3) as diagp,
            tc.tile_pool(name="outp", bufs=2) as outp,
            tc.tile_pool(name="ptr", bufs=2, space="PSUM") as ptr,
            tc.tile_pool(name="phh", bufs=2, space="PSUM") as phh,
            tc.tile_pool(name="psmall", bufs=2, space="PSUM") as psmall,
            tc.tile_pool(name="po", bufs=2, space="PSUM") as po,
        ):
            # ---- constants ----
            watt_sb = consts.tile([128, 8, 128], f32r)
            nc.sync.dma_start(
                out=watt_sb, in_=watt.ap().rearrange("(c p) a -> p c a", p=128)
            )
            wfull_sb = consts.tile([128, 1], f32)
            nc.sync.dma_start(out=wfull_sb, in_=wfull.ap())
            batt_sb = consts.tile([1, 128], f32)
            nc.sync.dma_start(out=batt_sb, in_=batt.ap())
            ident_sb = consts.tile([128, 128], f32r)
            ident18_sb = consts.tile([18, 18], f32)
            nc.sync.dma_start(out=ident18_sb, in_=ident18.ap())
            nc.sync.dma_start(out=ident_sb, in_=ident.ap())
            diag01_sb = consts.tile([128, 64], f32)
            nc.sync.dma_start(out=diag01_sb, in_=diag01.ap())
            i64_sb = consts.tile([128, 64], f32r)
            nc.sync.dma_start(out=i64_sb, in_=i64.ap())
            onesr_sb = consts.tile([1, 128], f32)
            nc.sync.dma_start(out=onesr_sb, in_=ones_row.ap())
            frame_sb = consts.tile([128, 2, 512], f32r)
            nc.sync.dma_start(
                out=frame_sb,
                in_=frame.ap()
                .rearrange("t b d -> (t b) d")
                .rearrange("(c p) d -> p c d", p=128),
            )

            # ---- preamble: frame^T and fproj = frame @ Wf + b_att ----
            frameT_sb = consts.tile([128, 4, 256], f32r)
            for J in range(4):
                pt = ptr.tile([128, 512], f32, tag="ptr", name=f"ptf{J}")
                for rc in range(2):
                    nc.tensor.transpose(
                        r(pt[:, rc * 128 : (rc + 1) * 128]),
                        frame_sb[:, rc, J * 128 : (J + 1) * 128],
                        ident_sb,
                    )
                nc.vector.tensor_copy(out=frameT_sb[:, J, :], in_=pt[:, :256])
            fproj_sb = consts.tile([128, 2, 128], f32r)
            for rc in range(2):
                pf = phh.tile([128, 512], f32, tag="phh", name=f"pfp{rc}")
                for J in range(4):
                    nc.tensor.matmul(
                        pf[:, :128],
                        lhsT=frameT_sb[:, J, rc * 128 : (rc + 1) * 128],
                        rhs=watt_sb[:, 4 + J, :],
                        start=(J == 0),
                        stop=False,
                    )
                nc.tensor.matmul(
                    pf[:, :128], lhsT=onesr_sb, rhs=batt_sb, start=False, stop=True
                )
                nc.scalar.copy(out=fproj_sb[:, rc, :], in_=pf[:, :128])

            # ---- per-timestep body ----
            def body(_iv=None):
                for t in range(T_LOC):
                    rnat = rnatp.tile([128, NCH, 512], f32r, tag="rnat", name=f"rn{t}")
                    nc.sync.dma_start(
                        out=rnat,
                        in_=region.ap()[t]
                        .rearrange("n b d -> (n b) d")
                        .rearrange("(c p) d -> p c d", p=128),
                    )
                    masku8 = smallp.tile(
                        [18, 128], mybir.dt.uint8, tag="masku8", name=f"mu{t}"
                    )
                    nc.sync.dma_start(
                        out=masku8,
                        in_=mask.ap()[t]
                        .rearrange("n b -> (n b)")
                        .rearrange("(c p) -> c p", p=128),
                    )
                    maskf = smallp.tile([18, 128], f32, tag="maskf", name=f"mf{t}")
                    # keep = 1 - mask
                    nc.scalar.activation(
                        out=maskf, in_=masku8, func=AF.Identity, bias=1.0, scale=-1.0
                    )
                    pmt_t = psmall.tile([128, 18], f32, tag="s", name=f"pm{t}")
                    nc.tensor.transpose(pmt_t, maskf, ident18_sb)
                    maskk = smallp.tile([128, 18], f32, tag="maskk", name=f"mk{t}")
                    nc.vector.tensor_copy(out=maskk, in_=pmt_t)

                    patt_t = psmall.tile([128, 18], f32, tag="s", name=f"pa{t}")
                    for g, (c0, cw) in enumerate(GROUPS):
                        nch_g = cw // 128
                        ph_g = phh.tile([128, 512], f32, tag="phh", name=f"ph{t}_{g}")
                        for J in range(4):
                            pt = ptr.tile(
                                [128, 512], f32, tag="ptr", name=f"pt{t}_{g}_{J}"
                            )
                            for cl in range(nch_g):
                                c = c0 // 128 + cl
                                nc.tensor.transpose(
                                    r(pt[:, cl * 128 : (cl + 1) * 128]),
                                    rnat[:, c, J * 128 : (J + 1) * 128],
                                    ident_sb,
                                )
                            rt = rtp.tile(
                                [128, 512], f32r, tag="rt", name=f"rt{t}_{g}_{J}"
                            )
                            if J % 2 == 0:
                                nc.vector.tensor_copy(out=rt[:, :cw], in_=pt[:, :cw])
                            else:
                                nc.scalar.copy(out=rt[:, :cw], in_=pt[:, :cw])
                            nc.tensor.matmul(
                                ph_g[:, :cw],
                                lhsT=watt_sb[:, J, :],
                                rhs=rt[:, :cw],
                                start=(J == 0),
                                stop=False,
                            )
                        # bias: fproj rows for this t against tiled I64
                        # (i64 input holds eye(64) duplicated on both
                        # partition halves so odd t can match base 64)
                        reps = cw // 64
                        rlo = (t % 2) * 64
                        i64h = i64_sb[rlo : rlo + 64, :]
                        i64b = bass.AP(
                            tensor=i64h.tensor,
                            offset=i64h.offset,
                            ap=[list(i64h.ap[0]), [0, reps], list(i64h.ap[1])],
                        )
                        nc.tensor.matmul(
                            ph_g[:, :cw],
                            lhsT=fproj_sb[rlo : rlo + 64, t // 2, :],
                            rhs=i64b,
                            start=False,
                            stop=True,
                        )
                        th = tanhp.tile([128, 512], f32, tag="th", name=f"th{t}_{g}")
                        nc.scalar.activation(
                            out=th[:, :cw], in_=ph_g[:, :cw], func=AF.Tanh
                        )
                        for cl in range(nch_g):
                            c = c0 // 128 + cl
                            nc.tensor.matmul(
                                patt_t[:, c : c + 1],
                                lhsT=th[:, cl * 128 : (cl + 1) * 128],
                                rhs=wfull_sb,
                                start=True,
                                stop=True,
                            )
                    expr = smallp.tile([128, 18], f32, tag="expr", name=f"ex{t}")
                    nc.scalar.activation(out=expr, in_=patt_t, func=AF.Exp)
                    expm = smallp.tile([128, 18], f32, tag="expm", name=f"em{t}")
                    sacc = smallp.tile([128, 1], f32, tag="sacc", name=f"sa{t}")
                    nc.vector.tensor_mul(expm, expr, maskk)
                    nc.vector.tensor_reduce(
                        out=sacc,
                        in_=expm,
                        axis=mybir.AxisListType.X,
                        op=mybir.AluOpType.add,
                    )
                    # fold partition-pairs: S[b] = sacc[b] + sacc[64+b] on PE
                    ps64 = psmall.tile([64, 1], f32, tag="s", name=f"ps{t}")
                    nc.tensor.matmul(
                        ps64, lhsT=diag01_sb, rhs=sacc, start=True, stop=True
                    )
                    rs = smallp.tile([64, 1], f32, tag="rs", name=f"rs{t}")
                    nc.vector.reciprocal(out=rs, in_=ps64)

                    po_t = po.tile([64, 512], f32, tag="po", name=f"po{t}")
                    for c in range(NCH):
                        dg = diagp.tile([128, 64], f32r, tag="dg", name=f"dg{t}_{c}")
                        nc.vector.tensor_scalar_mul(
                            out=dg, in0=diag01_sb, scalar1=expm[:, c : c + 1]
                        )
                        nc.tensor.matmul(
                            po_t,
                            lhsT=dg,
                            rhs=rnat[:, c, :],
                            start=(c == 0),
                            stop=(c == NCH - 1),
                        )
                    osb = outp.tile([64, 512], f32, tag="osb", name=f"ob{t}")
                    nc.vector.tensor_scalar_mul(out=osb, in0=po_t, scalar1=rs)
                    nc.sync.dma_start(out=out.ap()[t], in_=osb)

            if iters == 1:
                body()
            else:
                with tc.For_i(0, iters, 1) as iv:
                    body(iv)

    nc.compile()
    return nc


def _get_nc(iters=1):
    if iters not in _NC_CACHE:
        _NC_CACHE[iters] = _build_nc(iters)
    return _NC_CACHE[iters]


def _make_in_maps(region_feat, frame_feat, mask, W_att, b_att, W_full):
    diag01 = np.zeros((128, 64), np.float32)
    diag01[np.arange(128), np.arange(128) % 64] = 1.0
    consts = {
        "watt": np.ascontiguousarray(W_att, np.float32),
        "wfull": np.ascontiguousarray(W_full.reshape(A, 1), np.float32),
        "batt": np.ascontiguousarray(b_att.reshape(1, A), np.float32),
        "ident": np.eye(128, dtype=np.float32),
        "ident18": np.eye(18, dtype=np.float32),
        "diag01": diag01,
        "i64": np.vstack(
            [np.eye(64, dtype=np.float32), np.eye(64, dtype=np.float32)]
        ),
        "ones_row": np.ones((1, 128), np.float32),
    }
    in_maps = []
    for c in range(N_CORES):
        sl = slice(c * T_LOC, (c + 1) * T_LOC)
        in_maps.append(
            {
                "region": np.ascontiguousarray(region_feat[sl], np.float32),
                "frame": np.ascontiguousarray(frame_feat[sl], np.float32),
                "mask": np.ascontiguousarray(mask[sl]).astype(np.uint8),
                **consts,
            }
        )
    return in_maps


def kernel(region_feat, frame_feat, mask, W_att, b_att, W_full, b_full=None):
    """Full-input entry point. b_full is accepted but unused: softmax is
    invariant to a constant shift of the logits."""
    from concourse.bass_utils import run_bass_kernel_spmd

    region_feat = np.asarray(region_feat, np.float32)
    frame_feat = np.asarray(frame_feat, np.float32)
    mask = np.asarray(mask)
    nc = _get_nc()
    in_maps = _make_in_maps(region_feat, frame_feat, mask, W_att, b_att, W_full)
    res = run_bass_kernel_spmd(nc, in_maps, core_ids=list(range(N_CORES)))
    return np.concatenate(
        [res.results[c]["out"] for c in range(N_CORES)], axis=0
    ).astype(np.float32)
